# revision 1
# baseline (speedup 1.0000x reference)
"""Trainium2 Bass kernel v2 for GQA sliding-window causal self-attention.

Sharding: 8 cores = 2 batches x 4 kv-head groups. Per core: qkv projection
(4 q heads + 1 kv head + gate column), value-embed gate, RoPE, RMS-norm,
sliding-window attention, partial output projection. Host sums 4 bf16
partials per batch.

All matmuls run in bf16 (full PE rate). Softmax exp is split between the
Scalar engine (table exp) and DVE (Schraudolph bit-trick exp) to balance
engine load; only full (unmasked) blocks may take the approximate path.
"""

import functools
import sys
from contextlib import ExitStack

import numpy as np
import ml_dtypes

for _p in ("/opt/trn_rl_repo",):
    if _p not in sys.path:
        sys.path.insert(0, _p)

import concourse.bass as bass
import concourse.bacc as bacc
import concourse.mybir as mybir
import concourse.tile as tile
from concourse import bass_utils, library_config
from concourse.bass import ds, ts

F32 = mybir.dt.float32
BF16 = mybir.dt.bfloat16
I16 = mybir.dt.int16
I32 = mybir.dt.int32
AF = mybir.ActivationFunctionType
OP = mybir.AluOpType

B, T, C = 2, 2048, 1024
NH, NKV, HD = 16, 4, 64
QT = 512
KB = 128
NT = T // 128          # 16
NQ = T // QT           # 4
NQT = NT // NQ         # 4
KC = C // 128          # 8
LN_EPS = 1e-6
RMS_MUL = 1.2

# Schraudolph bf16 exp: i16 = rn(x*ASCH + BSCH); bitcast i16 -> bf16.
# Includes the 1/8 attention scale. Max rel err ~3.3%, softmax-common-mode
# bias cancels. Applied only to mask-free full blocks.
ASCH = (128.0 / np.log(2.0)) * 0.125
BSCH = 16250.5
# number of full-block exp ops (per (block, pair)) sent to DVE instead of ACT
SCHRAUD_N = 10
# fraction of output-projection evacuations handled by ACT (rest DVE)
MAGIC_RSQRT = float(np.frombuffer(np.uint32(0x5F3759DF).tobytes(),
                                  dtype=np.float32)[0])


def block_spans(q0, W, Tt):
    """Active k-blocks for q-tile [q0, q0+QT): list of
    (kb, L, R, tri_causal(c0, j0, w) | None, tri_window(c0, j0, w) | None)."""
    kb_lo = max(0, q0 - W) // KB
    kb_hi = min(Tt - 1, q0 + QT - 1) // KB
    out = []
    for kb in range(kb_lo, kb_hi + 1):
        k0 = kb * KB
        L = min(max(k0 - q0, 0), QT)
        R = min(max(k0 + W + KB - q0, 0), QT)
        if R <= L:
            continue
        tc_ = None
        c_lo = max(k0 - q0, 0)
        c_hi = min(k0 - q0 + KB - 1, QT - 1)
        if c_lo <= c_hi and k0 + KB - 1 > q0 + c_lo:
            tc_ = (c_lo, (q0 + c_lo) - k0, c_hi - c_lo + 1)
        tw = None
        w_lo = max(k0 + W + 1 - q0, 0)
        w_hi = min(k0 + W + KB - 1 - q0, QT - 1)
        if w_lo <= w_hi:
            tw = (w_lo, (q0 + w_lo) - k0 - W, w_hi - w_lo + 1)
        out.append((kb, L, R, tc_, tw))
    return out


def emit_kernel(tc, outs, ins, window):
    nc = tc.nc
    xT = ins["xT"].rearrange("(ko p) t -> p ko t", p=128)        # [128,8,T]
    W_all = ins["W_all"].rearrange("(ko p) m -> p ko m", p=128)  # [128,8,386]
    ve3 = ins["ve3"].rearrange("(n p) d -> p n d", p=128)        # [128,16,64]
    cosd = ins["cos"].rearrange("(n p) d -> p n d", p=128)       # [128,16,32]
    sind = ins["sin"].rearrange("(n p) d -> p n d", p=128)
    Wp = ins["Wproj"].rearrange("(ko p) n -> p ko n", p=128)     # [128,2,1024]
    masks = ins["masks"]                                         # [128,256]
    ident = ins["ident"]                                         # [128,128]
    out = outs["out"].rearrange("(n p) c -> n p c", p=128)       # [16,128,1024]

    stack = ExitStack()
    nc.gpsimd.load_library(library_config.proxy)

    const = stack.enter_context(tc.tile_pool(name="const", bufs=1))
    work = stack.enter_context(tc.tile_pool(name="work", bufs=1))

    W_sb = [const.tile([128, 1, 386], BF16, tag=f"w{k}", name=f"W_sb{k}")
            for k in range(KC)]
    xT_sb = [const.tile([128, 1, T], BF16, tag=f"x{k}", name=f"xT_sb{k}")
             for k in range(KC)]
    ve_sb = const.tile([128, NT, 64], BF16)
    cos_sb = const.tile([128, NT, 32], BF16)
    sin_sb = const.tile([128, NT, 32], BF16)
    Wp_sb = const.tile([128, 2, 1024], BF16)
    mask_sb = const.tile([128, 256], BF16)
    id_sb = const.tile([128, 128], BF16)
    magic_sb = const.tile([128, 20], F32)

    # spread the startup-critical loads (weights + quarter-0 x slices)
    # across engine DMA queues so the first projection isn't serialized
    # behind one queue
    qs = [nc.sync, nc.scalar, nc.gpsimd]
    for k in range(KC):
        qs[k % 3].dma_start(W_sb[k][:, 0, :], W_all[:, k, :])
    for k in range(KC):
        qs[(k + 1) % 3].dma_start(xT_sb[k][:, 0, ts(0, QT)],
                                  xT[:, k, ts(0, QT)])
    for j in range(1, NQ):
        for k in range(KC):
            nc.sync.dma_start(xT_sb[k][:, 0, ts(j, QT)],
                              xT[:, k, ts(j, QT)])
    nc.sync.dma_start(ve_sb[:], ve3[:])
    nc.sync.dma_start(cos_sb[:], cosd[:])
    nc.sync.dma_start(sin_sb[:], sind[:])
    nc.sync.dma_start(Wp_sb[:], Wp[:])
    nc.sync.dma_start(mask_sb[:], masks[:])
    nc.sync.dma_start(id_sb[:], ident[:])
    nc.gpsimd.memset(magic_sb[:], MAGIC_RSQRT)

    # persistent intermediates
    kT = work.tile([128, NT, 128], BF16)          # transposed k (dup rows)
    v_g = work.tile([128, NT, 65], BF16)          # v + ones column
    qT = [work.tile([128, 2, QT], BF16, name=f"qT{j}") for j in range(NQ)]
    yT = [work.tile([128, 2, QT], BF16, name=f"yT{j}") for j in range(NQ)]
    ss_q = [work.tile([128, NQT, 5], F32, name=f"ss{j}") for j in range(NQ)]
    ssc = [work.tile([128, NQT, 5], F32, name=f"ssc{j}") for j in range(NQ)]
    z_q = [work.tile([128, NQT], F32, name=f"zq{j}") for j in range(NQ)]
    g_q = [work.tile([128, NQT], F32, name=f"gq{j}") for j in range(NQ)]

    nc.gpsimd.memset(v_g[:, :, 64:65], 1.0)

    ps = stack.enter_context(tc.tile_pool(name="ps", bufs=1, space="PSUM"))
    sbp = stack.enter_context(tc.tile_pool(name="sbp", bufs=1))

    spans = {}
    for q0 in range(0, T, QT):
        sp = block_spans(q0, window, T)
        sp.sort(key=lambda s: (-(s[2] - s[1]), s[1]))
        spans[q0] = sp

    # assign Schraudolph (DVE) exp to full blocks, preferring later q-tiles
    # (ACT is busiest when attention is widest)
    schraud = set()
    budget = SCHRAUD_N
    for q0 in sorted(spans, reverse=True):
        for (kb, L, R, tcau, twin) in spans[q0]:
            if budget <= 0:
                break
            if tcau is None and twin is None:
                for pair in range(2):
                    if budget > 0:
                        schraud.add((q0, kb, pair))
                        budget -= 1

    nw_tmp = [None]

    def newton_rsqrt(mq, outq):
        """outq = 1/sqrt(mq) elementwise on [128, 20] f32 (DVE only)."""
        if nw_tmp[0] is None:
            nw_tmp[0] = [work.tile([128, 20], F32, name=f"nw{i}")
                         for i in range(3)]
        y0, y2, tt_ = nw_tmp[0]
        nc.vector.tensor_scalar(out=y0[:].bitcast(I32), in0=mq.bitcast(I32),
                                scalar1=1, scalar2=None,
                                op0=OP.arith_shift_right)
        nc.vector.tensor_tensor(y0[:].bitcast(I32), magic_sb[:].bitcast(I32),
                                y0[:].bitcast(I32), OP.subtract)
        for _ in range(2):
            nc.vector.tensor_tensor(y2[:], y0[:], y0[:], OP.mult)
            nc.vector.tensor_tensor(tt_[:], mq, y2[:], OP.mult)
            nc.vector.tensor_scalar(out=tt_[:], in0=tt_[:], scalar1=-0.5,
                                    scalar2=1.5, op0=OP.mult, op1=OP.add)
            nc.vector.tensor_tensor(y0[:], y0[:], tt_[:], OP.mult)
        nc.vector.tensor_copy(out=outq, in_=y0[:])

    # ---- task closures; emission order is interleaved so the PE always
    # ---- has independent filler work between dependent attention blocks.
    rp_keep = {}   # ti -> (qkv_sb, rp)

    def proj_tile(jq, tl):
        ti = jq * NQT + tl
        qkv_ps = ps.tile([128, 512], F32, tag="b1", bufs=2, name="qkv_ps")
        for k in range(KC):
            nc.tensor.matmul(
                qkv_ps[:, 0:386],
                xT_sb[k][:, 0, ts(ti, 128)],
                W_sb[k][:, 0, :],
                start=(k == 0), stop=(k == KC - 1),
            )
        qkv_sb = sbp.tile([128, 385], BF16, tag="qkvsb", bufs=6,
                          name="qkv_sb")
        nc.vector.tensor_copy(out=qkv_sb[:], in_=qkv_ps[:, 0:385])
        nc.vector.tensor_copy(out=z_q[jq][:, tl:tl + 1],
                              in_=qkv_ps[:, 384:385])

        qk = qkv_sb[:, 0:320].rearrange("p (h d) -> p h d", d=64)
        cos_b = cos_sb[:, ti, None, :].to_broadcast((128, 5, 32))
        sin_b = sin_sb[:, ti, None, :].to_broadcast((128, 5, 32))
        rp = sbp.tile([128, 5, 64], BF16, tag="rope", bufs=6, name="rp")
        t1 = sbp.tile([128, 5, 64], BF16, tag="ropetmp", bufs=2, name="t1")
        nc.vector.tensor_tensor(rp[:, :, 0:32], qk[:, :, 0:32], cos_b,
                                OP.mult)
        nc.vector.tensor_tensor(rp[:, :, 32:64], qk[:, :, 32:64], cos_b,
                                OP.mult)
        nc.vector.tensor_tensor(t1[:, :, 0:32], qk[:, :, 32:64], sin_b,
                                OP.mult)
        nc.vector.tensor_tensor(t1[:, :, 32:64], qk[:, :, 0:32], sin_b,
                                OP.mult)
        nc.gpsimd.tensor_tensor(rp[:, :, 0:32], rp[:, :, 0:32],
                                t1[:, :, 0:32], OP.add)
        nc.gpsimd.tensor_tensor(rp[:, :, 32:64], rp[:, :, 32:64],
                                t1[:, :, 32:64], OP.subtract)
        sq = sbp.tile([128, 5, 64], BF16, tag="sq", bufs=2, name="sq")
        nc.vector.tensor_tensor(sq[:], rp[:], rp[:], OP.mult)
        nc.vector.tensor_reduce(ss_q[jq][:, tl, :], sq[:],
                                mybir.AxisListType.X, OP.add)
        rp_keep[ti] = (qkv_sb, rp)

    def epi_head(jq):
        # rms scale via fast-inverse-sqrt + gate sigmoid for the quarter
        mq = sbp.tile([128, 20], F32, tag="mq", bufs=2, name="mq")
        nc.vector.tensor_scalar(
            out=mq[:], in0=ss_q[jq][:].rearrange("p a b -> p (a b)"),
            scalar1=1.0 / (64.0 * RMS_MUL * RMS_MUL),
            scalar2=LN_EPS / (RMS_MUL * RMS_MUL), op0=OP.mult, op1=OP.add)
        newton_rsqrt(mq[:], ssc[jq][:].rearrange("p a b -> p (a b)"))
        nc.scalar.activation(g_q[jq][:], z_q[jq][:], AF.Exp, scale=-1.0)
        nc.vector.tensor_scalar(out=g_q[jq][:], in0=g_q[jq][:], scalar1=1.0,
                                scalar2=None, op0=OP.add)
        nc.vector.reciprocal(g_q[jq][:], g_q[jq][:])

    def epi_tile(jq, tl):
        ti = jq * NQT + tl
        qkv_sb, rp = rp_keep.pop(ti)
        nc.vector.scalar_tensor_tensor(
            out=v_g[:, ti, 0:64], in0=ve_sb[:, ti, :],
            scalar=g_q[jq][:, tl:tl + 1], in1=qkv_sb[:, 320:384],
            op0=OP.mult, op1=OP.add)
        qkn = sbp.tile([128, 5, 64], BF16, tag="qkn", bufs=2, name="qkn")
        nc.gpsimd.tensor_tensor(
            qkn[:], rp[:],
            ssc[jq][:, tl, :, None].to_broadcast((128, 5, 64)), OP.mult)
        tr = ps.tile([128, 1024], BF16, tag="b1", bufs=2, name="tr")
        nc.tensor.transpose(
            tr[:, 0:128], qkn[:, 0:2, :].rearrange("p h d -> p (h d)"),
            id_sb[:])
        nc.tensor.transpose(
            tr[:, 128:256], qkn[:, 2:4, :].rearrange("p h d -> p (h d)"),
            id_sb[:])
        nc.tensor.transpose(tr[0:64, 256:384], qkn[:, 4, :], id_sb[:])
        nc.tensor.transpose(tr[64:128, 256:384], qkn[:, 4, :], id_sb[:])
        nc.vector.tensor_copy(
            out=qT[jq][:, :, ts(tl, 128)],
            in_=tr[:, 0:256].rearrange("p (h t) -> p h t", t=128))
        nc.scalar.copy(out=kT[:, ti, :], in_=tr[:, 256:384])

    y_live = {}    # pair -> [y_ps h0, y_ps h1]

    def attn_block(jq, pair, bi):
        q0 = jq * QT
        sp = spans[q0]
        nblk = len(sp)
        kb, L, R, tcau, twin = sp[bi]
        if bi == 0:
            y_live[pair] = [ps.tile([128, QT], F32, tag="y", bufs=2,
                                    name=f"y{h}") for h in range(2)]
        y_ps = y_live[pair]
        s_ps = ps.tile([128, 2, QT], F32, tag="s", bufs=2, name="s_ps")
        for h in range(2):
            rows = slice(64 * h, 64 * (h + 1))
            nc.tensor.matmul(
                s_ps[:, h, L:R],
                kT[rows, kb, :],
                qT[jq][rows, pair, L:R],
                start=True, stop=True,
                tile_position=(64 * h, 0),
            )
        p_sb = sbp.tile([128, 2, QT], BF16, tag="p", bufs=4, name="p_sb")
        if (q0, kb, pair) in schraud:
            nc.vector.tensor_scalar(
                out=p_sb[:, :, L:R].bitcast(I16),
                in0=s_ps[:, :, L:R], scalar1=ASCH, scalar2=BSCH,
                op0=OP.mult, op1=OP.add)
        else:
            nc.scalar.activation(p_sb[:, :, L:R], s_ps[:, :, L:R],
                                 AF.Exp, scale=0.125)
        for trip, moff_base in ((tcau, 0), (twin, 128)):
            if trip is None:
                continue
            c0, j0, w = trip
            # window triangles on the lightly-loaded gpsimd engine,
            # causal ones on DVE (all operands SBUF-resident)
            eng = nc.gpsimd if moff_base == 128 else nc.vector
            eng.tensor_tensor(
                p_sb[:, :, c0:c0 + w],
                p_sb[:, :, c0:c0 + w],
                mask_sb[:, None, moff_base + j0:moff_base + j0 + w]
                .to_broadcast((128, 2, w)),
                OP.mult)
        for h in range(2):
            nc.tensor.matmul(
                y_ps[h][0:65, L:R],
                v_g[:, kb, :],
                p_sb[:, h, L:R],
                start=(bi == 0),
                stop=(bi == nblk - 1),
            )

    def norm_pair(jq, pair):
        y_ps = y_live.pop(pair)
        for h in range(2):
            row_sb = sbp.tile([1, QT], F32, tag="srow", bufs=2,
                              name="row_sb")
            nc.scalar.copy(out=row_sb[:], in_=y_ps[h][64:65, :])
            r_sb = sbp.tile([1, QT], F32, tag="rrow", bufs=2, name="r_sb")
            nc.vector.reciprocal_approx_fast(out=r_sb[:], in_=row_sb[:])
            rbc = sbp.tile([64, QT], F32, tag="rbc", bufs=2, name="rbc")
            nc.gpsimd.partition_broadcast(rbc[:], r_sb[:])
            nc.vector.tensor_tensor(
                yT[jq][64 * h:64 * (h + 1), pair, :],
                y_ps[h][0:64, :], rbc[:], OP.mult)

    def outp_half(jq, tl, n):
        ti = jq * NQT + tl
        tsl = ts(tl, 128)
        nsl = ts(n, 512)
        o_ps = ps.tile([128, 512], F32, tag="b1", bufs=2, name="o_ps")
        nc.tensor.matmul(o_ps[:], yT[jq][:, 0, tsl],
                         Wp_sb[:, 0, nsl], start=True, stop=False)
        nc.tensor.matmul(o_ps[:], yT[jq][:, 1, tsl],
                         Wp_sb[:, 1, nsl], start=False, stop=True)
        o_sb = sbp.tile([128, 512], BF16, tag="osb", bufs=3, name="o_sb")
        if n == 0:
            nc.scalar.copy(out=o_sb[:], in_=o_ps[:])
        else:
            nc.vector.tensor_copy(out=o_sb[:], in_=o_ps[:])
        nc.sync.dma_start(out[ti][:, nsl], o_sb[:])

    # ---- emission schedule ----
    for tl in range(NQT):
        proj_tile(0, tl)
    epi_head(0)
    for tl in range(NQT):
        epi_tile(0, tl)

    for jq in range(NQ):
        sp = spans[jq * QT]
        blocks = [(pair, bi) for pair in range(2) for bi in range(len(sp))]
        fillers = []
        for tl in range(NQT):
            fillers.append(lambda tl=tl: outp_half(jq - 1, tl, 0) or
                           outp_half(jq - 1, tl, 1))
        if jq + 1 < NQ:
            for tl in range(NQT):
                fillers.append(lambda tl=tl: proj_tile(jq + 1, tl))
            fillers.append(lambda: epi_head(jq + 1))
            for tl in range(NQT):
                fillers.append(lambda tl=tl: epi_tile(jq + 1, tl))
        if jq == 0:
            fillers = fillers[NQT:]  # no previous quarter to project out
        # distribute fillers evenly across the block stream
        fi = 0
        nb, nf = len(blocks), len(fillers)
        for i, (pair, bi) in enumerate(blocks):
            attn_block(jq, pair, bi)
            if bi == len(sp) - 1:
                norm_pair(jq, pair)
            want = (i + 1) * nf // nb
            while fi < want:
                fillers[fi]()
                fi += 1
        while fi < len(fillers):
            fillers[fi]()
            fi += 1
    for tl in range(NQT):
        outp_half(NQ - 1, tl, 0)
        outp_half(NQ - 1, tl, 1)

    stack.close()


@functools.lru_cache(maxsize=4)
def _build(window):
    nc = bacc.Bacc("TRN2", target_bir_lowering=False, debug=False,
                   enable_asserts=False, num_devices=8)
    ins = {
        "xT": nc.dram_tensor("xT", [C, T], BF16, kind="ExternalInput").ap(),
        "W_all": nc.dram_tensor("W_all", [C, 386], BF16,
                                kind="ExternalInput").ap(),
        "ve3": nc.dram_tensor("ve3", [T, 64], BF16, kind="ExternalInput").ap(),
        "cos": nc.dram_tensor("cos", [T, 32], BF16, kind="ExternalInput").ap(),
        "sin": nc.dram_tensor("sin", [T, 32], BF16, kind="ExternalInput").ap(),
        "Wproj": nc.dram_tensor("Wproj", [256, 1024], BF16,
                                kind="ExternalInput").ap(),
        "masks": nc.dram_tensor("masks", [128, 256], BF16,
                                kind="ExternalInput").ap(),
        "ident": nc.dram_tensor("ident", [128, 128], BF16,
                                kind="ExternalInput").ap(),
    }
    outs = {
        "out": nc.dram_tensor("out", [T, C], BF16, kind="ExternalOutput").ap(),
    }
    with tile.TileContext(nc) as tc:
        emit_kernel(tc, outs, ins, window)
    nc.compile()
    return nc


def host_constants():
    m_c = (np.arange(KB)[:, None] <= np.arange(KB)[None, :]).astype(np.float32)
    m_w = (np.arange(KB)[:, None] >= np.arange(KB)[None, :]).astype(np.float32)
    masks = np.concatenate([m_c, m_w], axis=1)
    ident = np.eye(128, dtype=np.float32)
    return masks, ident


def _bf(a):
    return np.asarray(a, dtype=ml_dtypes.bfloat16)


def prep_core_inputs(core, x, ve, cos, sin, Wq, Wk, Wv, Wproj, Wg):
    """Host-side shard prep for one core. core = b*4 + g."""
    b, g = divmod(core, NKV)
    masks, ident = host_constants()
    W_allf = np.zeros((C, 386), np.float32)
    W_allf[:, 0:256] = Wq[:, g * 256:(g + 1) * 256]
    W_allf[:, 256:320] = Wk[:, g * 64:(g + 1) * 64]
    W_allf[:, 320:384] = Wv[:, g * 64:(g + 1) * 64]
    W_allf[:12, 384] = Wg[:, g]
    return {
        "xT": _bf(np.ascontiguousarray(x[b].T)),
        "W_all": _bf(W_allf),
        "ve3": _bf(3.0 * ve[b, :, g * 64:(g + 1) * 64]),
        "cos": _bf(cos.reshape(T, 32)),
        "sin": _bf(sin.reshape(T, 32)),
        "Wproj": _bf(np.ascontiguousarray(Wproj[g * 256:(g + 1) * 256, :])),
        "masks": _bf(masks),
        "ident": _bf(ident),
    }


TRACE = False
LAST_EXEC_NS = None
LAST_TRACE = None


def kernel(x, ve, cos, sin, Wq, Wk, Wv, Wproj, Wg, window_size):
    x = np.asarray(x, np.float32)
    ve = np.asarray(ve, np.float32)
    cos = np.asarray(cos, np.float32)
    sin = np.asarray(sin, np.float32)
    Wq = np.asarray(Wq, np.float32)
    Wk = np.asarray(Wk, np.float32)
    Wv = np.asarray(Wv, np.float32)
    Wproj = np.asarray(Wproj, np.float32)
    Wg = np.asarray(Wg, np.float32)
    window = int(np.asarray(window_size))

    nc = _build(window)
    in_maps = [
        prep_core_inputs(core, x, ve, cos, sin, Wq, Wk, Wv, Wproj, Wg)
        for core in range(8)
    ]
    res = bass_utils.run_bass_kernel_spmd(nc, in_maps, core_ids=list(range(8)),
                                          trace=TRACE)
    if TRACE:
        global LAST_EXEC_NS, LAST_TRACE
        LAST_EXEC_NS = res.exec_time_ns
        LAST_TRACE = res
    out = np.zeros((B, T, C), np.float32)
    for core in range(8):
        out[core // NKV] += np.asarray(res.results[core]["out"], np.float32)
    return out



# revision 9
# speedup vs baseline: 1.0599x; 1.0599x over previous
"""Trainium2 Bass kernel v2 for GQA sliding-window causal self-attention.

Sharding: 8 cores = 2 batches x 4 kv-head groups. Per core: qkv projection
(4 q heads + 1 kv head + gate column), value-embed gate, RoPE, RMS-norm,
sliding-window attention, partial output projection. Host sums 4 bf16
partials per batch.

All matmuls run in bf16 (full PE rate). Softmax exp is split between the
Scalar engine (table exp) and DVE (Schraudolph bit-trick exp) to balance
engine load; only full (unmasked) blocks may take the approximate path.
"""

import functools
import sys
from contextlib import ExitStack

import numpy as np
import ml_dtypes

for _p in ("/opt/trn_rl_repo",):
    if _p not in sys.path:
        sys.path.insert(0, _p)

import concourse.bass as bass
import concourse.bacc as bacc
import concourse.mybir as mybir
import concourse.tile as tile
from concourse import bass_utils, library_config
from concourse.bass import ds, ts

F32 = mybir.dt.float32
BF16 = mybir.dt.bfloat16
I16 = mybir.dt.int16
I32 = mybir.dt.int32
AF = mybir.ActivationFunctionType
OP = mybir.AluOpType

B, T, C = 2, 2048, 1024
NH, NKV, HD = 16, 4, 64
QT = 512
KB = 128
NT = T // 128          # 16
NQ = T // QT           # 4
NQT = NT // NQ         # 4
KC = C // 128          # 8
LN_EPS = 1e-6
RMS_MUL = 1.2

# Schraudolph bf16 exp: i16 = rn(x*ASCH + BSCH); bitcast i16 -> bf16.
# Includes the 1/8 attention scale. Max rel err ~3.3%, softmax-common-mode
# bias cancels. Applied only to mask-free full blocks.
ASCH = (128.0 / np.log(2.0)) * 0.125
BSCH = 16250.5
# number of full-block exp ops (per (block, pair)) sent to DVE instead of ACT
SCHRAUD_N = 10
# fraction of output-projection evacuations handled by ACT (rest DVE)
MAGIC_RSQRT = float(np.frombuffer(np.uint32(0x5F3759DF).tobytes(),
                                  dtype=np.float32)[0])


def block_spans(q0, W, Tt):
    """Active k-blocks for q-tile [q0, q0+QT): list of
    (kb, L, R, tri_causal(c0, j0, w) | None, tri_window(c0, j0, w) | None)."""
    kb_lo = max(0, q0 - W) // KB
    kb_hi = min(Tt - 1, q0 + QT - 1) // KB
    out = []
    for kb in range(kb_lo, kb_hi + 1):
        k0 = kb * KB
        L = min(max(k0 - q0, 0), QT)
        R = min(max(k0 + W + KB - q0, 0), QT)
        if R <= L:
            continue
        tc_ = None
        c_lo = max(k0 - q0, 0)
        c_hi = min(k0 - q0 + KB - 1, QT - 1)
        if c_lo <= c_hi and k0 + KB - 1 > q0 + c_lo:
            tc_ = (c_lo, (q0 + c_lo) - k0, c_hi - c_lo + 1)
        tw = None
        w_lo = max(k0 + W + 1 - q0, 0)
        w_hi = min(k0 + W + KB - 1 - q0, QT - 1)
        if w_lo <= w_hi:
            tw = (w_lo, (q0 + w_lo) - k0 - W, w_hi - w_lo + 1)
        out.append((kb, L, R, tc_, tw))
    return out


def emit_kernel(tc, outs, ins, window):
    nc = tc.nc
    xT = ins["xT"].rearrange("(ko p) t -> p ko t", p=128)        # [128,8,T]
    W_all = ins["W_all"].rearrange("(ko p) m -> p ko m", p=128)  # [128,8,386]
    ve3 = ins["ve3"].rearrange("(n p) d -> p n d", p=128)        # [128,16,64]
    cosd = ins["cos"].rearrange("(n p) d -> p n d", p=128)       # [128,16,32]
    sind = ins["sin"].rearrange("(n p) d -> p n d", p=128)
    Wp = ins["Wproj"].rearrange("(ko p) n -> p ko n", p=128)     # [128,2,1024]
    masks = ins["masks"]                                         # [128,256]
    ident = ins["ident"]                                         # [128,128]
    out = outs["out"].rearrange("(n p) c -> n p c", p=128)       # [16,128,1024]

    stack = ExitStack()
    nc.gpsimd.load_library(library_config.proxy)

    const = stack.enter_context(tc.tile_pool(name="const", bufs=1))
    work = stack.enter_context(tc.tile_pool(name="work", bufs=1))

    W_sb = const.tile([128, KC, 386], BF16)
    xT_sb = const.tile([128, KC, T], BF16)
    ve_sb = const.tile([128, NT, 64], BF16)
    cos_sb = const.tile([128, NT, 32], BF16)
    sin_sb = const.tile([128, NT, 32], BF16)
    Wp_sb = const.tile([128, 2, 1024], BF16)
    mask_sb = const.tile([128, 256], BF16)
    id_sb = const.tile([128, 128], BF16)
    magic_sb = const.tile([128, 20], F32)

    # DMA dispatches cost ~600ns each on the issuing engine, and the gpsimd
    # engine is blocked ~12us at startup by load_library - so the
    # startup-critical loads (W, x) go on sync/scalar in need-order at k
    # granularity, and only late-needed tensors ride on gpsimd.
    nc.sync.dma_start(W_sb[:, 0:2, :], W_all[:, 0:2, :])
    nc.sync.dma_start(xT_sb[:, 0, :], xT[:, 0, :])
    nc.sync.dma_start(xT_sb[:, 1, :], xT[:, 1, :])
    nc.sync.dma_start(xT_sb[:, 2, :], xT[:, 2, :])
    nc.sync.dma_start(xT_sb[:, 3, :], xT[:, 3, :])
    nc.scalar.dma_start(id_sb[:], ident[:])
    nc.scalar.dma_start(W_sb[:, 2:KC, :], W_all[:, 2:KC, :])
    nc.scalar.dma_start(xT_sb[:, 4, :], xT[:, 4, :])
    nc.scalar.dma_start(xT_sb[:, 5, :], xT[:, 5, :])
    nc.scalar.dma_start(xT_sb[:, 6, :], xT[:, 6, :])
    nc.scalar.dma_start(xT_sb[:, 7, :], xT[:, 7, :])
    nc.scalar.dma_start(cos_sb[:], cosd[:])
    nc.scalar.dma_start(sin_sb[:], sind[:])
    nc.gpsimd.dma_start(ve_sb[:], ve3[:])
    nc.gpsimd.dma_start(mask_sb[:], masks[:])
    nc.gpsimd.dma_start(Wp_sb[:], Wp[:])
    nc.gpsimd.memset(magic_sb[:], MAGIC_RSQRT)

    # persistent intermediates
    kT = work.tile([128, NT, 128], BF16)          # transposed k (dup rows)
    v_g = work.tile([128, NT, 65], BF16)          # v + ones column
    qT = [work.tile([128, 2, QT], BF16, name=f"qT{j}") for j in range(NQ)]
    yT = [work.tile([128, 2, QT], BF16, name=f"yT{j}") for j in range(NQ)]
    ss_q = [work.tile([128, NQT, 5], F32, name=f"ss{j}") for j in range(NQ)]
    ssc = [work.tile([128, NQT, 5], F32, name=f"ssc{j}") for j in range(NQ)]
    z_q = [work.tile([128, NQT], F32, name=f"zq{j}") for j in range(NQ)]
    g_q = [work.tile([128, NQT], F32, name=f"gq{j}") for j in range(NQ)]

    nc.gpsimd.memset(v_g[:, :, 64:65], 1.0)

    ps = stack.enter_context(tc.tile_pool(name="ps", bufs=1, space="PSUM"))
    sbp = stack.enter_context(tc.tile_pool(name="sbp", bufs=1))

    spans = {}
    for q0 in range(0, T, QT):
        sp = block_spans(q0, window, T)
        sp.sort(key=lambda s: (-(s[2] - s[1]), s[1]))
        spans[q0] = sp

    # assign Schraudolph (DVE) exp to full blocks, preferring later q-tiles
    # (ACT is busiest when attention is widest)
    schraud = set()
    budget = SCHRAUD_N
    for q0 in sorted(spans, reverse=True):
        for (kb, L, R, tcau, twin) in spans[q0]:
            if budget <= 0:
                break
            if tcau is None and twin is None:
                for pair in range(2):
                    if budget > 0:
                        schraud.add((q0, kb, pair))
                        budget -= 1

    nw_tmp = [None]

    def newton_rsqrt(mq, outq):
        """outq = 1/sqrt(mq) elementwise on [128, 20] f32 (DVE only)."""
        if nw_tmp[0] is None:
            nw_tmp[0] = [work.tile([128, 20], F32, name=f"nw{i}")
                         for i in range(3)]
        y0, y2, tt_ = nw_tmp[0]
        nc.vector.tensor_scalar(out=y0[:].bitcast(I32), in0=mq.bitcast(I32),
                                scalar1=1, scalar2=None,
                                op0=OP.arith_shift_right)
        nc.vector.tensor_tensor(y0[:].bitcast(I32), magic_sb[:].bitcast(I32),
                                y0[:].bitcast(I32), OP.subtract)
        for _ in range(2):
            nc.vector.tensor_tensor(y2[:], y0[:], y0[:], OP.mult)
            nc.vector.tensor_tensor(tt_[:], mq, y2[:], OP.mult)
            nc.vector.tensor_scalar(out=tt_[:], in0=tt_[:], scalar1=-0.5,
                                    scalar2=1.5, op0=OP.mult, op1=OP.add)
            nc.vector.tensor_tensor(y0[:], y0[:], tt_[:], OP.mult)
        nc.vector.tensor_copy(out=outq, in_=y0[:])

    # ---- task closures; emission order is interleaved so the PE always
    # ---- has independent filler work between dependent attention blocks.
    rp_keep = {}   # ti -> (qkv_sb, rp)

    def proj_tile(jq, tl):
        ti = jq * NQT + tl
        qkv_ps = ps.tile([128, 512], F32, tag="b1", bufs=2, name="qkv_ps")
        for k in range(KC):
            nc.tensor.matmul(
                qkv_ps[:, 0:386],
                xT_sb[:, k, ts(ti, 128)],
                W_sb[:, k, :],
                start=(k == 0), stop=(k == KC - 1),
            )
        qkv_sb = sbp.tile([128, 385], BF16, tag="qkvsb", bufs=6,
                          name="qkv_sb")
        nc.vector.tensor_copy(out=qkv_sb[:], in_=qkv_ps[:, 0:385])
        nc.vector.tensor_copy(out=z_q[jq][:, tl:tl + 1],
                              in_=qkv_ps[:, 384:385])

        qk = qkv_sb[:, 0:320].rearrange("p (h d) -> p h d", d=64)
        cos_b = cos_sb[:, ti, None, :].to_broadcast((128, 5, 32))
        sin_b = sin_sb[:, ti, None, :].to_broadcast((128, 5, 32))
        rp = sbp.tile([128, 5, 64], BF16, tag="rope", bufs=6, name="rp")
        t1 = sbp.tile([128, 5, 64], BF16, tag="ropetmp", bufs=2, name="t1")
        nc.vector.tensor_tensor(rp[:, :, 0:32], qk[:, :, 0:32], cos_b,
                                OP.mult)
        nc.vector.tensor_tensor(rp[:, :, 32:64], qk[:, :, 32:64], cos_b,
                                OP.mult)
        nc.vector.tensor_tensor(t1[:, :, 0:32], qk[:, :, 32:64], sin_b,
                                OP.mult)
        nc.vector.tensor_tensor(t1[:, :, 32:64], qk[:, :, 0:32], sin_b,
                                OP.mult)
        nc.gpsimd.tensor_tensor(rp[:, :, 0:32], rp[:, :, 0:32],
                                t1[:, :, 0:32], OP.add)
        nc.gpsimd.tensor_tensor(rp[:, :, 32:64], rp[:, :, 32:64],
                                t1[:, :, 32:64], OP.subtract)
        sq = sbp.tile([128, 5, 64], BF16, tag="sq", bufs=2, name="sq")
        nc.vector.tensor_tensor(sq[:], rp[:], rp[:], OP.mult)
        nc.vector.tensor_reduce(ss_q[jq][:, tl, :], sq[:],
                                mybir.AxisListType.X, OP.add)
        rp_keep[ti] = (qkv_sb, rp)

    def epi_head(jq):
        # rms scale via fast-inverse-sqrt + gate sigmoid for the quarter
        mq = sbp.tile([128, 20], F32, tag="mq", bufs=2, name="mq")
        nc.vector.tensor_scalar(
            out=mq[:], in0=ss_q[jq][:].rearrange("p a b -> p (a b)"),
            scalar1=1.0 / (64.0 * RMS_MUL * RMS_MUL),
            scalar2=LN_EPS / (RMS_MUL * RMS_MUL), op0=OP.mult, op1=OP.add)
        newton_rsqrt(mq[:], ssc[jq][:].rearrange("p a b -> p (a b)"))
        nc.scalar.activation(g_q[jq][:], z_q[jq][:], AF.Exp, scale=-1.0)
        nc.vector.tensor_scalar(out=g_q[jq][:], in0=g_q[jq][:], scalar1=1.0,
                                scalar2=None, op0=OP.add)
        nc.vector.reciprocal(g_q[jq][:], g_q[jq][:])

    def epi_tile(jq, tl):
        ti = jq * NQT + tl
        qkv_sb, rp = rp_keep.pop(ti)
        nc.vector.scalar_tensor_tensor(
            out=v_g[:, ti, 0:64], in0=ve_sb[:, ti, :],
            scalar=g_q[jq][:, tl:tl + 1], in1=qkv_sb[:, 320:384],
            op0=OP.mult, op1=OP.add)
        qkn = sbp.tile([128, 5, 64], BF16, tag="qkn", bufs=2, name="qkn")
        nc.gpsimd.tensor_tensor(
            qkn[:], rp[:],
            ssc[jq][:, tl, :, None].to_broadcast((128, 5, 64)), OP.mult)
        tr = ps.tile([128, 1024], BF16, tag="b1", bufs=2, name="tr")
        nc.tensor.transpose(
            tr[:, 0:128], qkn[:, 0:2, :].rearrange("p h d -> p (h d)"),
            id_sb[:])
        nc.tensor.transpose(
            tr[:, 128:256], qkn[:, 2:4, :].rearrange("p h d -> p (h d)"),
            id_sb[:])
        nc.tensor.transpose(tr[0:64, 256:384], qkn[:, 4, :], id_sb[:])
        nc.tensor.transpose(tr[64:128, 256:384], qkn[:, 4, :], id_sb[:])
        nc.vector.tensor_copy(
            out=qT[jq][:, :, ts(tl, 128)],
            in_=tr[:, 0:256].rearrange("p (h t) -> p h t", t=128))
        nc.scalar.copy(out=kT[:, ti, :], in_=tr[:, 256:384])

    y_live = {}    # pair -> [y_ps h0, y_ps h1]

    def attn_block(jq, pair, bi):
        q0 = jq * QT
        sp = spans[q0]
        nblk = len(sp)
        kb, L, R, tcau, twin = sp[bi]
        if bi == 0:
            y_live[pair] = [ps.tile([128, QT], F32, tag="y", bufs=2,
                                    name=f"y{h}") for h in range(2)]
        y_ps = y_live[pair]
        s_ps = ps.tile([128, 2, QT], F32, tag="s", bufs=2, name="s_ps")
        for h in range(2):
            rows = slice(64 * h, 64 * (h + 1))
            nc.tensor.matmul(
                s_ps[:, h, L:R],
                kT[rows, kb, :],
                qT[jq][rows, pair, L:R],
                start=True, stop=True,
                tile_position=(64 * h, 0),
            )
        p_sb = sbp.tile([128, 2, QT], BF16, tag="p", bufs=4, name="p_sb")
        if (q0, kb, pair) in schraud:
            nc.vector.tensor_scalar(
                out=p_sb[:, :, L:R].bitcast(I16),
                in0=s_ps[:, :, L:R], scalar1=ASCH, scalar2=BSCH,
                op0=OP.mult, op1=OP.add)
        else:
            nc.scalar.activation(p_sb[:, :, L:R], s_ps[:, :, L:R],
                                 AF.Exp, scale=0.125)
        for trip, moff_base in ((tcau, 0), (twin, 128)):
            if trip is None:
                continue
            c0, j0, w = trip
            # window triangles on the lightly-loaded gpsimd engine,
            # causal ones on DVE (all operands SBUF-resident)
            eng = nc.gpsimd if moff_base == 128 else nc.vector
            eng.tensor_tensor(
                p_sb[:, :, c0:c0 + w],
                p_sb[:, :, c0:c0 + w],
                mask_sb[:, None, moff_base + j0:moff_base + j0 + w]
                .to_broadcast((128, 2, w)),
                OP.mult)
        for h in range(2):
            nc.tensor.matmul(
                y_ps[h][0:65, L:R],
                v_g[:, kb, :],
                p_sb[:, h, L:R],
                start=(bi == 0),
                stop=(bi == nblk - 1),
            )

    def norm_pair(jq, pair):
        y_ps = y_live.pop(pair)
        for h in range(2):
            row_sb = sbp.tile([1, QT], F32, tag="srow", bufs=2,
                              name="row_sb")
            nc.scalar.copy(out=row_sb[:], in_=y_ps[h][64:65, :])
            r_sb = sbp.tile([1, QT], F32, tag="rrow", bufs=2, name="r_sb")
            nc.vector.reciprocal_approx_fast(out=r_sb[:], in_=row_sb[:])
            rbc = sbp.tile([64, QT], F32, tag="rbc", bufs=2, name="rbc")
            nc.gpsimd.partition_broadcast(rbc[:], r_sb[:])
            nc.vector.tensor_tensor(
                yT[jq][64 * h:64 * (h + 1), pair, :],
                y_ps[h][0:64, :], rbc[:], OP.mult)

    def outp_half(jq, tl, n):
        ti = jq * NQT + tl
        tsl = ts(tl, 128)
        nsl = ts(n, 512)
        o_ps = ps.tile([128, 512], F32, tag="b1", bufs=2, name="o_ps")
        nc.tensor.matmul(o_ps[:], yT[jq][:, 0, tsl],
                         Wp_sb[:, 0, nsl], start=True, stop=False)
        nc.tensor.matmul(o_ps[:], yT[jq][:, 1, tsl],
                         Wp_sb[:, 1, nsl], start=False, stop=True)
        o_sb = sbp.tile([128, 512], BF16, tag="osb", bufs=3, name="o_sb")
        if n == 0:
            nc.scalar.copy(out=o_sb[:], in_=o_ps[:])
        else:
            nc.vector.tensor_copy(out=o_sb[:], in_=o_ps[:])
        nc.sync.dma_start(out[ti][:, nsl], o_sb[:])

    # ---- emission schedule: proj runs two quarters ahead, epi one ----
    for tl in range(NQT):
        proj_tile(0, tl)
    for tl in range(NQT):
        proj_tile(1, tl)
    epi_head(0)
    for tl in range(NQT):
        epi_tile(0, tl)

    for jq in range(NQ):
        sp = spans[jq * QT]
        blocks = [(pair, bi) for pair in range(2) for bi in range(len(sp))]
        fillers = []
        if jq > 0:
            for tl in range(NQT):
                fillers.append(lambda tl=tl: outp_half(jq - 1, tl, 0) or
                               outp_half(jq - 1, tl, 1))
        if jq + 1 < NQ:
            fillers.append(lambda: epi_head(jq + 1))
            for tl in range(NQT):
                fillers.append(lambda tl=tl: epi_tile(jq + 1, tl))
        if jq + 2 < NQ:
            for tl in range(NQT):
                fillers.append(lambda tl=tl: proj_tile(jq + 2, tl))
        # distribute fillers evenly across the block stream
        fi = 0
        nb, nf = len(blocks), len(fillers)
        for i, (pair, bi) in enumerate(blocks):
            attn_block(jq, pair, bi)
            if bi == len(sp) - 1:
                norm_pair(jq, pair)
            want = (i + 1) * nf // nb
            while fi < want:
                fillers[fi]()
                fi += 1
        while fi < len(fillers):
            fillers[fi]()
            fi += 1
    for tl in range(NQT):
        outp_half(NQ - 1, tl, 0)
        outp_half(NQ - 1, tl, 1)

    stack.close()


@functools.lru_cache(maxsize=4)
def _build(window):
    nc = bacc.Bacc("TRN2", target_bir_lowering=False, debug=False,
                   enable_asserts=False, num_devices=8)
    ins = {
        "xT": nc.dram_tensor("xT", [C, T], BF16, kind="ExternalInput").ap(),
        "W_all": nc.dram_tensor("W_all", [C, 386], BF16,
                                kind="ExternalInput").ap(),
        "ve3": nc.dram_tensor("ve3", [T, 64], BF16, kind="ExternalInput").ap(),
        "cos": nc.dram_tensor("cos", [T, 32], BF16, kind="ExternalInput").ap(),
        "sin": nc.dram_tensor("sin", [T, 32], BF16, kind="ExternalInput").ap(),
        "Wproj": nc.dram_tensor("Wproj", [256, 1024], BF16,
                                kind="ExternalInput").ap(),
        "masks": nc.dram_tensor("masks", [128, 256], BF16,
                                kind="ExternalInput").ap(),
        "ident": nc.dram_tensor("ident", [128, 128], BF16,
                                kind="ExternalInput").ap(),
    }
    outs = {
        "out": nc.dram_tensor("out", [T, C], BF16, kind="ExternalOutput").ap(),
    }
    with tile.TileContext(nc) as tc:
        emit_kernel(tc, outs, ins, window)
    nc.compile()
    return nc


def host_constants():
    m_c = (np.arange(KB)[:, None] <= np.arange(KB)[None, :]).astype(np.float32)
    m_w = (np.arange(KB)[:, None] >= np.arange(KB)[None, :]).astype(np.float32)
    masks = np.concatenate([m_c, m_w], axis=1)
    ident = np.eye(128, dtype=np.float32)
    return masks, ident


def _bf(a):
    return np.asarray(a, dtype=ml_dtypes.bfloat16)


def prep_core_inputs(core, x, ve, cos, sin, Wq, Wk, Wv, Wproj, Wg):
    """Host-side shard prep for one core. core = b*4 + g."""
    b, g = divmod(core, NKV)
    masks, ident = host_constants()
    W_allf = np.zeros((C, 386), np.float32)
    W_allf[:, 0:256] = Wq[:, g * 256:(g + 1) * 256]
    W_allf[:, 256:320] = Wk[:, g * 64:(g + 1) * 64]
    W_allf[:, 320:384] = Wv[:, g * 64:(g + 1) * 64]
    W_allf[:12, 384] = Wg[:, g]
    return {
        "xT": _bf(np.ascontiguousarray(x[b].T)),
        "W_all": _bf(W_allf),
        "ve3": _bf(3.0 * ve[b, :, g * 64:(g + 1) * 64]),
        "cos": _bf(cos.reshape(T, 32)),
        "sin": _bf(sin.reshape(T, 32)),
        "Wproj": _bf(np.ascontiguousarray(Wproj[g * 256:(g + 1) * 256, :])),
        "masks": _bf(masks),
        "ident": _bf(ident),
    }


TRACE = False
LAST_EXEC_NS = None
LAST_TRACE = None


def kernel(x, ve, cos, sin, Wq, Wk, Wv, Wproj, Wg, window_size):
    x = np.asarray(x, np.float32)
    ve = np.asarray(ve, np.float32)
    cos = np.asarray(cos, np.float32)
    sin = np.asarray(sin, np.float32)
    Wq = np.asarray(Wq, np.float32)
    Wk = np.asarray(Wk, np.float32)
    Wv = np.asarray(Wv, np.float32)
    Wproj = np.asarray(Wproj, np.float32)
    Wg = np.asarray(Wg, np.float32)
    window = int(np.asarray(window_size))

    nc = _build(window)
    in_maps = [
        prep_core_inputs(core, x, ve, cos, sin, Wq, Wk, Wv, Wproj, Wg)
        for core in range(8)
    ]
    res = bass_utils.run_bass_kernel_spmd(nc, in_maps, core_ids=list(range(8)),
                                          trace=TRACE)
    if TRACE:
        global LAST_EXEC_NS, LAST_TRACE
        LAST_EXEC_NS = res.exec_time_ns
        LAST_TRACE = res
    out = np.zeros((B, T, C), np.float32)
    for core in range(8):
        out[core // NKV] += np.asarray(res.results[core]["out"], np.float32)
    return out



# revision 10
# speedup vs baseline: 1.0978x; 1.0358x over previous
"""Trainium2 Bass kernel v2 for GQA sliding-window causal self-attention.

Sharding: 8 cores = 2 batches x 4 kv-head groups. Per core: qkv projection
(4 q heads + 1 kv head + gate column), value-embed gate, RoPE, RMS-norm,
sliding-window attention, partial output projection. Host sums 4 bf16
partials per batch.

All matmuls run in bf16 (full PE rate). Softmax exp is split between the
Scalar engine (table exp) and DVE (Schraudolph bit-trick exp) to balance
engine load; only full (unmasked) blocks may take the approximate path.
"""

import functools
import sys
from contextlib import ExitStack

import numpy as np
import ml_dtypes

for _p in ("/opt/trn_rl_repo",):
    if _p not in sys.path:
        sys.path.insert(0, _p)

import concourse.bass as bass
import concourse.bacc as bacc
import concourse.mybir as mybir
import concourse.tile as tile
from concourse import bass_utils, library_config
from concourse.bass import ds, ts

F32 = mybir.dt.float32
BF16 = mybir.dt.bfloat16
I16 = mybir.dt.int16
I32 = mybir.dt.int32
AF = mybir.ActivationFunctionType
OP = mybir.AluOpType

B, T, C = 2, 2048, 1024
NH, NKV, HD = 16, 4, 64
QT = 512
KB = 128
NT = T // 128          # 16
NQ = T // QT           # 4
NQT = NT // NQ         # 4
KC = C // 128          # 8
LN_EPS = 1e-6
RMS_MUL = 1.2

# Schraudolph bf16 exp: i16 = rn(x*ASCH + BSCH); bitcast i16 -> bf16.
# Includes the 1/8 attention scale. Max rel err ~3.3%, softmax-common-mode
# bias cancels. Applied only to mask-free full blocks.
ASCH = (128.0 / np.log(2.0)) * 0.125
BSCH = 16250.5
# number of full-block exp ops (per (block, pair)) sent to DVE instead of ACT
SCHRAUD_N = 10
# fraction of output-projection evacuations handled by ACT (rest DVE)
MAGIC_RSQRT = float(np.frombuffer(np.uint32(0x5F3759DF).tobytes(),
                                  dtype=np.float32)[0])


def block_spans(q0, W, Tt):
    """Active k-blocks for q-tile [q0, q0+QT): list of
    (kb, L, R, tri_causal(c0, j0, w) | None, tri_window(c0, j0, w) | None)."""
    kb_lo = max(0, q0 - W) // KB
    kb_hi = min(Tt - 1, q0 + QT - 1) // KB
    out = []
    for kb in range(kb_lo, kb_hi + 1):
        k0 = kb * KB
        L = min(max(k0 - q0, 0), QT)
        R = min(max(k0 + W + KB - q0, 0), QT)
        if R <= L:
            continue
        tc_ = None
        c_lo = max(k0 - q0, 0)
        c_hi = min(k0 - q0 + KB - 1, QT - 1)
        if c_lo <= c_hi and k0 + KB - 1 > q0 + c_lo:
            tc_ = (c_lo, (q0 + c_lo) - k0, c_hi - c_lo + 1)
        tw = None
        w_lo = max(k0 + W + 1 - q0, 0)
        w_hi = min(k0 + W + KB - 1 - q0, QT - 1)
        if w_lo <= w_hi:
            tw = (w_lo, (q0 + w_lo) - k0 - W, w_hi - w_lo + 1)
        out.append((kb, L, R, tc_, tw))
    return out


def emit_kernel(tc, outs, ins, window):
    nc = tc.nc
    xT = ins["xT"].rearrange("(ko p) t -> p ko t", p=128)        # [128,8,T]
    W_all = ins["W_all"].rearrange("(ko p) m -> p ko m", p=128)  # [128,8,386]
    ve3 = ins["ve3"].rearrange("(n p) d -> p n d", p=128)        # [128,16,64]
    cosd = ins["cos"].rearrange("(n p) d -> p n d", p=128)       # [128,16,32]
    sind = ins["sin"].rearrange("(n p) d -> p n d", p=128)
    Wp = ins["Wproj"].rearrange("(ko p) n -> p ko n", p=128)     # [128,2,1024]
    masks = ins["masks"]                                         # [128,256]
    ident = ins["ident"]                                         # [128,128]
    out = outs["out"].rearrange("(n p) c -> n p c", p=128)       # [16,128,1024]

    stack = ExitStack()
    nc.gpsimd.load_library(library_config.proxy)

    const = stack.enter_context(tc.tile_pool(name="const", bufs=1))
    work = stack.enter_context(tc.tile_pool(name="work", bufs=1))

    W_sb = const.tile([128, KC, 386], BF16)
    xT_sb = const.tile([128, KC, T], BF16)
    ve_sb = const.tile([128, NT, 64], BF16)
    cos_sb = const.tile([128, NT, 32], BF16)
    sin_sb = const.tile([128, NT, 32], BF16)
    Wp_sb = const.tile([128, 2, 1024], BF16)
    mask_sb = const.tile([128, 256], BF16)
    id_sb = const.tile([128, 128], BF16)
    magic_sb = const.tile([128, 20], F32)

    # DMA dispatches cost ~600ns each on the issuing engine, and the gpsimd
    # engine is blocked ~12us at startup by load_library - so the
    # startup-critical loads (W, x) go on sync/scalar in need-order at k
    # granularity, and only late-needed tensors ride on gpsimd.
    nc.sync.dma_start(W_sb[:, 0:2, :], W_all[:, 0:2, :])
    nc.sync.dma_start(xT_sb[:, 0, ts(0, QT)], xT[:, 0, ts(0, QT)])
    nc.sync.dma_start(xT_sb[:, 1, ts(0, QT)], xT[:, 1, ts(0, QT)])
    nc.sync.dma_start(W_sb[:, 2:4, :], W_all[:, 2:4, :])
    nc.sync.dma_start(xT_sb[:, 2, ts(0, QT)], xT[:, 2, ts(0, QT)])
    nc.sync.dma_start(xT_sb[:, 3, ts(0, QT)], xT[:, 3, ts(0, QT)])
    nc.sync.dma_start(xT_sb[:, 0:2, QT:T], xT[:, 0:2, QT:T])
    nc.sync.dma_start(xT_sb[:, 2:4, QT:T], xT[:, 2:4, QT:T])
    nc.scalar.dma_start(id_sb[:], ident[:])
    nc.scalar.dma_start(xT_sb[:, 4, ts(0, QT)], xT[:, 4, ts(0, QT)])
    nc.scalar.dma_start(W_sb[:, 4:6, :], W_all[:, 4:6, :])
    nc.scalar.dma_start(xT_sb[:, 5, ts(0, QT)], xT[:, 5, ts(0, QT)])
    nc.scalar.dma_start(W_sb[:, 6:KC, :], W_all[:, 6:KC, :])
    nc.scalar.dma_start(xT_sb[:, 6, ts(0, QT)], xT[:, 6, ts(0, QT)])
    nc.scalar.dma_start(xT_sb[:, 7, ts(0, QT)], xT[:, 7, ts(0, QT)])
    nc.scalar.dma_start(cos_sb[:], cosd[:])
    nc.scalar.dma_start(sin_sb[:], sind[:])
    nc.scalar.dma_start(xT_sb[:, 4:6, QT:T], xT[:, 4:6, QT:T])
    nc.scalar.dma_start(xT_sb[:, 6:KC, QT:T], xT[:, 6:KC, QT:T])
    nc.gpsimd.dma_start(ve_sb[:], ve3[:])
    nc.gpsimd.dma_start(mask_sb[:], masks[:])
    nc.gpsimd.dma_start(Wp_sb[:], Wp[:])
    nc.gpsimd.memset(magic_sb[:], MAGIC_RSQRT)

    # persistent intermediates
    kT = work.tile([128, NT, 128], BF16)          # transposed k (dup rows)
    v_g = work.tile([128, NT, 65], BF16)          # v + ones column
    qT = [work.tile([128, 2, QT], BF16, name=f"qT{j}") for j in range(NQ)]
    yT = [work.tile([128, 2, QT], BF16, name=f"yT{j}") for j in range(NQ)]
    ss_q = [work.tile([128, NQT, 5], F32, name=f"ss{j}") for j in range(NQ)]
    ssc = [work.tile([128, NQT, 5], F32, name=f"ssc{j}") for j in range(NQ)]
    g_q = [work.tile([128, NQT], F32, name=f"gq{j}") for j in range(NQ)]

    nc.gpsimd.memset(v_g[:, :, 64:65], 1.0)

    ps = stack.enter_context(tc.tile_pool(name="ps", bufs=1, space="PSUM"))
    sbp = stack.enter_context(tc.tile_pool(name="sbp", bufs=1))

    spans = {}
    for q0 in range(0, T, QT):
        sp = block_spans(q0, window, T)
        sp.sort(key=lambda s: (-(s[2] - s[1]), s[1]))
        spans[q0] = sp

    # assign Schraudolph (DVE) exp to full blocks, preferring later q-tiles
    # (ACT is busiest when attention is widest)
    schraud = set()
    budget = SCHRAUD_N
    for q0 in sorted(spans, reverse=True):
        for (kb, L, R, tcau, twin) in spans[q0]:
            if budget <= 0:
                break
            if tcau is None and twin is None:
                for pair in range(2):
                    if budget > 0:
                        schraud.add((q0, kb, pair))
                        budget -= 1

    nw_tmp = [None]

    def newton_rsqrt(mq, outq):
        """outq = 1/sqrt(mq) elementwise on [128, 20] f32 (DVE only)."""
        if nw_tmp[0] is None:
            nw_tmp[0] = [work.tile([128, 20], F32, name=f"nw{i}")
                         for i in range(3)]
        y0, y2, tt_ = nw_tmp[0]
        nc.vector.tensor_scalar(out=y0[:].bitcast(I32), in0=mq.bitcast(I32),
                                scalar1=1, scalar2=None,
                                op0=OP.arith_shift_right)
        nc.vector.tensor_tensor(y0[:].bitcast(I32), magic_sb[:].bitcast(I32),
                                y0[:].bitcast(I32), OP.subtract)
        for _ in range(2):
            nc.vector.tensor_tensor(y2[:], y0[:], y0[:], OP.mult)
            nc.vector.tensor_tensor(tt_[:], mq, y2[:], OP.mult)
            nc.vector.tensor_scalar(out=tt_[:], in0=tt_[:], scalar1=-0.5,
                                    scalar2=1.5, op0=OP.mult, op1=OP.add)
            nc.vector.tensor_tensor(y0[:], y0[:], tt_[:], OP.mult)
        nc.vector.tensor_copy(out=outq, in_=y0[:])

    # ---- task closures; emission order is interleaved so the PE always
    # ---- has independent filler work between dependent attention blocks.
    rp_keep = {}   # ti -> (qkv_sb, rp)

    def proj_tile(jq, tl):
        ti = jq * NQT + tl
        qkv_ps = ps.tile([128, 512], F32, tag="b1", bufs=2, name="qkv_ps")
        for k in range(KC):
            nc.tensor.matmul(
                qkv_ps[:, 0:386],
                xT_sb[:, k, ts(ti, 128)],
                W_sb[:, k, :],
                start=(k == 0), stop=(k == KC - 1),
            )
        qkv_sb = sbp.tile([128, 385], BF16, tag="qkvsb", bufs=6,
                          name="qkv_sb")
        nc.vector.tensor_copy(out=qkv_sb[:], in_=qkv_ps[:, 0:385])
        nc.scalar.activation(g_q[jq][:, tl:tl + 1], qkv_sb[:, 384:385],
                             AF.Exp, scale=-1.0)

        qk = qkv_sb[:, 0:320].rearrange("p (h d) -> p h d", d=64)
        cos_b = cos_sb[:, ti, None, :].to_broadcast((128, 5, 32))
        sin_b = sin_sb[:, ti, None, :].to_broadcast((128, 5, 32))
        rp = sbp.tile([128, 5, 64], BF16, tag="rope", bufs=6, name="rp")
        t1 = sbp.tile([128, 5, 64], BF16, tag="ropetmp", bufs=2, name="t1")
        nc.vector.tensor_tensor(rp[:, :, 0:32], qk[:, :, 0:32], cos_b,
                                OP.mult)
        nc.vector.tensor_tensor(rp[:, :, 32:64], qk[:, :, 32:64], cos_b,
                                OP.mult)
        nc.vector.tensor_tensor(t1[:, :, 0:32], qk[:, :, 32:64], sin_b,
                                OP.mult)
        nc.vector.tensor_tensor(t1[:, :, 32:64], qk[:, :, 0:32], sin_b,
                                OP.mult)
        nc.gpsimd.tensor_tensor(rp[:, :, 0:32], rp[:, :, 0:32],
                                t1[:, :, 0:32], OP.add)
        nc.gpsimd.tensor_tensor(rp[:, :, 32:64], rp[:, :, 32:64],
                                t1[:, :, 32:64], OP.subtract)
        sq = sbp.tile([128, 5, 64], BF16, tag="sq", bufs=2, name="sq")
        nc.vector.tensor_tensor(sq[:], rp[:], rp[:], OP.mult)
        nc.vector.tensor_reduce(ss_q[jq][:, tl, :], sq[:],
                                mybir.AxisListType.X, OP.add)
        rp_keep[ti] = (qkv_sb, rp)

    def epi_head(jq):
        # rms scale via fast-inverse-sqrt + gate sigmoid for the quarter
        mq = sbp.tile([128, 20], F32, tag="mq", bufs=2, name="mq")
        nc.vector.tensor_scalar(
            out=mq[:], in0=ss_q[jq][:].rearrange("p a b -> p (a b)"),
            scalar1=1.0 / (64.0 * RMS_MUL * RMS_MUL),
            scalar2=LN_EPS / (RMS_MUL * RMS_MUL), op0=OP.mult, op1=OP.add)
        newton_rsqrt(mq[:], ssc[jq][:].rearrange("p a b -> p (a b)"))
        nc.vector.tensor_scalar(out=g_q[jq][:], in0=g_q[jq][:], scalar1=1.0,
                                scalar2=None, op0=OP.add)
        nc.vector.reciprocal(g_q[jq][:], g_q[jq][:])

    def epi_tile(jq, tl):
        ti = jq * NQT + tl
        qkv_sb, rp = rp_keep.pop(ti)
        nc.vector.scalar_tensor_tensor(
            out=v_g[:, ti, 0:64], in0=ve_sb[:, ti, :],
            scalar=g_q[jq][:, tl:tl + 1], in1=qkv_sb[:, 320:384],
            op0=OP.mult, op1=OP.add)
        qkn = sbp.tile([128, 5, 64], BF16, tag="qkn", bufs=2, name="qkn")
        nc.gpsimd.tensor_tensor(
            qkn[:], rp[:],
            ssc[jq][:, tl, :, None].to_broadcast((128, 5, 64)), OP.mult)
        tr = ps.tile([128, 1024], BF16, tag="b1", bufs=2, name="tr")
        nc.tensor.transpose(
            tr[:, 0:128], qkn[:, 0:2, :].rearrange("p h d -> p (h d)"),
            id_sb[:])
        nc.tensor.transpose(
            tr[:, 128:256], qkn[:, 2:4, :].rearrange("p h d -> p (h d)"),
            id_sb[:])
        nc.tensor.transpose(tr[0:64, 256:384], qkn[:, 4, :], id_sb[:])
        nc.tensor.transpose(tr[64:128, 256:384], qkn[:, 4, :], id_sb[:])
        nc.vector.tensor_copy(
            out=qT[jq][:, :, ts(tl, 128)],
            in_=tr[:, 0:256].rearrange("p (h t) -> p h t", t=128))
        nc.scalar.copy(out=kT[:, ti, :], in_=tr[:, 256:384])

    y_live = {}    # pair -> [y_ps h0, y_ps h1]

    def attn_block(jq, pair, bi):
        q0 = jq * QT
        sp = spans[q0]
        nblk = len(sp)
        kb, L, R, tcau, twin = sp[bi]
        if bi == 0:
            y_live[pair] = [ps.tile([128, QT], F32, tag="y", bufs=2,
                                    name=f"y{h}") for h in range(2)]
        y_ps = y_live[pair]
        s_ps = ps.tile([128, 2, QT], F32, tag="s", bufs=2, name="s_ps")
        for h in range(2):
            rows = slice(64 * h, 64 * (h + 1))
            nc.tensor.matmul(
                s_ps[:, h, L:R],
                kT[rows, kb, :],
                qT[jq][rows, pair, L:R],
                start=True, stop=True,
                tile_position=(64 * h, 0),
            )
        p_sb = sbp.tile([128, 2, QT], BF16, tag="p", bufs=4, name="p_sb")
        if (q0, kb, pair) in schraud:
            nc.vector.tensor_scalar(
                out=p_sb[:, :, L:R].bitcast(I16),
                in0=s_ps[:, :, L:R], scalar1=ASCH, scalar2=BSCH,
                op0=OP.mult, op1=OP.add)
        else:
            nc.scalar.activation(p_sb[:, :, L:R], s_ps[:, :, L:R],
                                 AF.Exp, scale=0.125)
        for trip, moff_base in ((tcau, 0), (twin, 128)):
            if trip is None:
                continue
            c0, j0, w = trip
            # window triangles on the lightly-loaded gpsimd engine,
            # causal ones on DVE (all operands SBUF-resident)
            eng = nc.gpsimd if moff_base == 128 else nc.vector
            eng.tensor_tensor(
                p_sb[:, :, c0:c0 + w],
                p_sb[:, :, c0:c0 + w],
                mask_sb[:, None, moff_base + j0:moff_base + j0 + w]
                .to_broadcast((128, 2, w)),
                OP.mult)
        for h in range(2):
            nc.tensor.matmul(
                y_ps[h][0:65, L:R],
                v_g[:, kb, :],
                p_sb[:, h, L:R],
                start=(bi == 0),
                stop=(bi == nblk - 1),
            )

    def norm_pair(jq, pair):
        y_ps = y_live.pop(pair)
        for h in range(2):
            row_sb = sbp.tile([1, QT], F32, tag="srow", bufs=2,
                              name="row_sb")
            nc.scalar.copy(out=row_sb[:], in_=y_ps[h][64:65, :])
            r_sb = sbp.tile([1, QT], F32, tag="rrow", bufs=2, name="r_sb")
            nc.vector.reciprocal_approx_fast(out=r_sb[:], in_=row_sb[:])
            rbc = sbp.tile([64, QT], F32, tag="rbc", bufs=2, name="rbc")
            nc.gpsimd.partition_broadcast(rbc[:], r_sb[:])
            nc.vector.tensor_tensor(
                yT[jq][64 * h:64 * (h + 1), pair, :],
                y_ps[h][0:64, :], rbc[:], OP.mult)

    def outp_half(jq, tl, n):
        ti = jq * NQT + tl
        tsl = ts(tl, 128)
        nsl = ts(n, 512)
        o_ps = ps.tile([128, 512], F32, tag="b1", bufs=2, name="o_ps")
        nc.tensor.matmul(o_ps[:], yT[jq][:, 0, tsl],
                         Wp_sb[:, 0, nsl], start=True, stop=False)
        nc.tensor.matmul(o_ps[:], yT[jq][:, 1, tsl],
                         Wp_sb[:, 1, nsl], start=False, stop=True)
        o_sb = sbp.tile([128, 512], BF16, tag="osb", bufs=3, name="o_sb")
        if n == 0:
            nc.scalar.copy(out=o_sb[:], in_=o_ps[:])
        else:
            nc.vector.tensor_copy(out=o_sb[:], in_=o_ps[:])
        nc.sync.dma_start(out[ti][:, nsl], o_sb[:])

    # ---- emission schedule: proj runs two quarters ahead, epi one ----
    for tl in range(NQT):
        proj_tile(0, tl)
    for tl in range(NQT):
        proj_tile(1, tl)
    epi_head(0)
    for tl in range(NQT):
        epi_tile(0, tl)

    for jq in range(NQ):
        sp = spans[jq * QT]
        blocks = [(pair, bi) for pair in range(2) for bi in range(len(sp))]
        fillers = []
        for tl in range(NQT):
            if jq + 1 < NQ and tl == 0:
                fillers.append(lambda: epi_head(jq + 1))
            if jq + 1 < NQ:
                fillers.append(lambda tl=tl: epi_tile(jq + 1, tl))
            if jq > 0:
                fillers.append(lambda tl=tl: outp_half(jq - 1, tl, 0) or
                               outp_half(jq - 1, tl, 1))
        if jq + 2 < NQ:
            for tl in range(NQT):
                fillers.append(lambda tl=tl: proj_tile(jq + 2, tl))
        # distribute fillers evenly across the block stream
        fi = 0
        nb, nf = len(blocks), len(fillers)
        for i, (pair, bi) in enumerate(blocks):
            attn_block(jq, pair, bi)
            if bi == len(sp) - 1:
                norm_pair(jq, pair)
            want = (i + 1) * nf // nb
            while fi < want:
                fillers[fi]()
                fi += 1
        while fi < len(fillers):
            fillers[fi]()
            fi += 1
    for tl in range(NQT):
        outp_half(NQ - 1, tl, 0)
        outp_half(NQ - 1, tl, 1)

    stack.close()


@functools.lru_cache(maxsize=4)
def _build(window):
    nc = bacc.Bacc("TRN2", target_bir_lowering=False, debug=False,
                   enable_asserts=False, num_devices=8)
    ins = {
        "xT": nc.dram_tensor("xT", [C, T], BF16, kind="ExternalInput").ap(),
        "W_all": nc.dram_tensor("W_all", [C, 386], BF16,
                                kind="ExternalInput").ap(),
        "ve3": nc.dram_tensor("ve3", [T, 64], BF16, kind="ExternalInput").ap(),
        "cos": nc.dram_tensor("cos", [T, 32], BF16, kind="ExternalInput").ap(),
        "sin": nc.dram_tensor("sin", [T, 32], BF16, kind="ExternalInput").ap(),
        "Wproj": nc.dram_tensor("Wproj", [256, 1024], BF16,
                                kind="ExternalInput").ap(),
        "masks": nc.dram_tensor("masks", [128, 256], BF16,
                                kind="ExternalInput").ap(),
        "ident": nc.dram_tensor("ident", [128, 128], BF16,
                                kind="ExternalInput").ap(),
    }
    outs = {
        "out": nc.dram_tensor("out", [T, C], BF16, kind="ExternalOutput").ap(),
    }
    with tile.TileContext(nc) as tc:
        emit_kernel(tc, outs, ins, window)
    nc.compile()
    return nc


def host_constants():
    m_c = (np.arange(KB)[:, None] <= np.arange(KB)[None, :]).astype(np.float32)
    m_w = (np.arange(KB)[:, None] >= np.arange(KB)[None, :]).astype(np.float32)
    masks = np.concatenate([m_c, m_w], axis=1)
    ident = np.eye(128, dtype=np.float32)
    return masks, ident


def _bf(a):
    return np.asarray(a, dtype=ml_dtypes.bfloat16)


def prep_core_inputs(core, x, ve, cos, sin, Wq, Wk, Wv, Wproj, Wg):
    """Host-side shard prep for one core. core = b*4 + g."""
    b, g = divmod(core, NKV)
    masks, ident = host_constants()
    W_allf = np.zeros((C, 386), np.float32)
    W_allf[:, 0:256] = Wq[:, g * 256:(g + 1) * 256]
    W_allf[:, 256:320] = Wk[:, g * 64:(g + 1) * 64]
    W_allf[:, 320:384] = Wv[:, g * 64:(g + 1) * 64]
    W_allf[:12, 384] = Wg[:, g]
    return {
        "xT": _bf(np.ascontiguousarray(x[b].T)),
        "W_all": _bf(W_allf),
        "ve3": _bf(3.0 * ve[b, :, g * 64:(g + 1) * 64]),
        "cos": _bf(cos.reshape(T, 32)),
        "sin": _bf(sin.reshape(T, 32)),
        "Wproj": _bf(np.ascontiguousarray(Wproj[g * 256:(g + 1) * 256, :])),
        "masks": _bf(masks),
        "ident": _bf(ident),
    }


TRACE = False
LAST_EXEC_NS = None
LAST_TRACE = None


def kernel(x, ve, cos, sin, Wq, Wk, Wv, Wproj, Wg, window_size):
    x = np.asarray(x, np.float32)
    ve = np.asarray(ve, np.float32)
    cos = np.asarray(cos, np.float32)
    sin = np.asarray(sin, np.float32)
    Wq = np.asarray(Wq, np.float32)
    Wk = np.asarray(Wk, np.float32)
    Wv = np.asarray(Wv, np.float32)
    Wproj = np.asarray(Wproj, np.float32)
    Wg = np.asarray(Wg, np.float32)
    window = int(np.asarray(window_size))

    nc = _build(window)
    in_maps = [
        prep_core_inputs(core, x, ve, cos, sin, Wq, Wk, Wv, Wproj, Wg)
        for core in range(8)
    ]
    res = bass_utils.run_bass_kernel_spmd(nc, in_maps, core_ids=list(range(8)),
                                          trace=TRACE)
    if TRACE:
        global LAST_EXEC_NS, LAST_TRACE
        LAST_EXEC_NS = res.exec_time_ns
        LAST_TRACE = res
    out = np.zeros((B, T, C), np.float32)
    for core in range(8):
        out[core // NKV] += np.asarray(res.results[core]["out"], np.float32)
    return out



# revision 11
# speedup vs baseline: 1.1109x; 1.0119x over previous
"""Trainium2 Bass kernel v2 for GQA sliding-window causal self-attention.

Sharding: 8 cores = 2 batches x 4 kv-head groups. Per core: qkv projection
(4 q heads + 1 kv head + gate column), value-embed gate, RoPE, RMS-norm,
sliding-window attention, partial output projection. Host sums 4 bf16
partials per batch.

All matmuls run in bf16 (full PE rate). Softmax exp is split between the
Scalar engine (table exp) and DVE (Schraudolph bit-trick exp) to balance
engine load; only full (unmasked) blocks may take the approximate path.
"""

import functools
import sys
from contextlib import ExitStack

import numpy as np
import ml_dtypes

for _p in ("/opt/trn_rl_repo",):
    if _p not in sys.path:
        sys.path.insert(0, _p)

import concourse.bass as bass
import concourse.bacc as bacc
import concourse.mybir as mybir
import concourse.tile as tile
from concourse import bass_utils, library_config
from concourse.bass import ds, ts

F32 = mybir.dt.float32
BF16 = mybir.dt.bfloat16
I16 = mybir.dt.int16
I32 = mybir.dt.int32
AF = mybir.ActivationFunctionType
OP = mybir.AluOpType

B, T, C = 2, 2048, 1024
NH, NKV, HD = 16, 4, 64
QT = 512
KB = 128
NT = T // 128          # 16
NQ = T // QT           # 4
NQT = NT // NQ         # 4
KC = C // 128          # 8
LN_EPS = 1e-6
RMS_MUL = 1.2

# Schraudolph bf16 exp: i16 = rn(x*ASCH + BSCH); bitcast i16 -> bf16.
# Includes the 1/8 attention scale. Max rel err ~3.3%, softmax-common-mode
# bias cancels. Applied only to mask-free full blocks.
ASCH = (128.0 / np.log(2.0)) * 0.125
BSCH = 16250.5
# number of full-block exp ops (per (block, pair)) sent to DVE instead of ACT
SCHRAUD_N = 10
# fraction of output-projection evacuations handled by ACT (rest DVE)
MAGIC_RSQRT = float(np.frombuffer(np.uint32(0x5F3759DF).tobytes(),
                                  dtype=np.float32)[0])


def block_spans(q0, W, Tt):
    """Active k-blocks for q-tile [q0, q0+QT): list of
    (kb, L, R, tri_causal(c0, j0, w) | None, tri_window(c0, j0, w) | None)."""
    kb_lo = max(0, q0 - W) // KB
    kb_hi = min(Tt - 1, q0 + QT - 1) // KB
    out = []
    for kb in range(kb_lo, kb_hi + 1):
        k0 = kb * KB
        L = min(max(k0 - q0, 0), QT)
        R = min(max(k0 + W + KB - q0, 0), QT)
        if R <= L:
            continue
        tc_ = None
        c_lo = max(k0 - q0, 0)
        c_hi = min(k0 - q0 + KB - 1, QT - 1)
        if c_lo <= c_hi and k0 + KB - 1 > q0 + c_lo:
            tc_ = (c_lo, (q0 + c_lo) - k0, c_hi - c_lo + 1)
        tw = None
        w_lo = max(k0 + W + 1 - q0, 0)
        w_hi = min(k0 + W + KB - 1 - q0, QT - 1)
        if w_lo <= w_hi:
            tw = (w_lo, (q0 + w_lo) - k0 - W, w_hi - w_lo + 1)
        out.append((kb, L, R, tc_, tw))
    return out


def emit_kernel(tc, outs, ins, window):
    nc = tc.nc
    xT = ins["xT"].rearrange("(ko p) t -> p ko t", p=128)        # [128,8,T]
    W_all = ins["W_all"].rearrange("(ko p) m -> p ko m", p=128)  # [128,8,386]
    ve3 = ins["ve3"].rearrange("(n p) d -> p n d", p=128)        # [128,16,64]
    cosd = ins["cos"].rearrange("(n p) d -> p n d", p=128)       # [128,16,32]
    sind = ins["sin"].rearrange("(n p) d -> p n d", p=128)
    Wp = ins["Wproj"].rearrange("(ko p) n -> p ko n", p=128)     # [128,2,1024]
    masks = ins["masks"]                                         # [128,256]
    ident = ins["ident"]                                         # [128,128]
    out = outs["out"].rearrange("(n p) c -> n p c", p=128)       # [16,128,1024]

    stack = ExitStack()
    nc.gpsimd.load_library(library_config.proxy)

    const = stack.enter_context(tc.tile_pool(name="const", bufs=1))
    work = stack.enter_context(tc.tile_pool(name="work", bufs=1))

    W_sb = const.tile([128, KC, 386], BF16)
    xT_sb = const.tile([128, KC, T], BF16)
    ve_sb = const.tile([128, NT, 64], BF16)
    cos_sb = const.tile([128, NT, 32], BF16)
    sin_sb = const.tile([128, NT, 32], BF16)
    Wp_sb = const.tile([128, 2, 1024], BF16)
    mask_sb = const.tile([128, 256], BF16)
    id_sb = const.tile([128, 128], BF16)
    magic_sb = const.tile([128, 20], F32)

    # DMA dispatches cost ~600ns each on the issuing engine, and the gpsimd
    # engine is blocked ~12us at startup by load_library - so the
    # startup-critical loads (W, x) go on sync/scalar in need-order at k
    # granularity, and only late-needed tensors ride on gpsimd.
    nc.sync.dma_start(xT_sb[:, 0, 0:256], xT[:, 0, 0:256])
    nc.sync.dma_start(W_sb[:, 0:1, :], W_all[:, 0:1, :])
    nc.sync.dma_start(xT_sb[:, 1, 0:256], xT[:, 1, 0:256])
    nc.sync.dma_start(W_sb[:, 1:2, :], W_all[:, 1:2, :])
    nc.sync.dma_start(xT_sb[:, 0, 256:QT], xT[:, 0, 256:QT])
    nc.sync.dma_start(xT_sb[:, 1, 256:QT], xT[:, 1, 256:QT])
    nc.sync.dma_start(W_sb[:, 2:4, :], W_all[:, 2:4, :])
    nc.sync.dma_start(xT_sb[:, 2, ts(0, QT)], xT[:, 2, ts(0, QT)])
    nc.sync.dma_start(xT_sb[:, 3, ts(0, QT)], xT[:, 3, ts(0, QT)])
    nc.sync.dma_start(xT_sb[:, 0:2, QT:T], xT[:, 0:2, QT:T])
    nc.sync.dma_start(xT_sb[:, 2:4, QT:T], xT[:, 2:4, QT:T])
    nc.scalar.dma_start(id_sb[:], ident[:])
    nc.scalar.dma_start(xT_sb[:, 4, ts(0, QT)], xT[:, 4, ts(0, QT)])
    nc.scalar.dma_start(W_sb[:, 4:6, :], W_all[:, 4:6, :])
    nc.scalar.dma_start(xT_sb[:, 5, ts(0, QT)], xT[:, 5, ts(0, QT)])
    nc.scalar.dma_start(W_sb[:, 6:KC, :], W_all[:, 6:KC, :])
    nc.scalar.dma_start(xT_sb[:, 6, ts(0, QT)], xT[:, 6, ts(0, QT)])
    nc.scalar.dma_start(xT_sb[:, 7, ts(0, QT)], xT[:, 7, ts(0, QT)])
    nc.scalar.dma_start(cos_sb[:], cosd[:])
    nc.scalar.dma_start(sin_sb[:], sind[:])
    nc.scalar.dma_start(xT_sb[:, 4:6, QT:T], xT[:, 4:6, QT:T])
    nc.scalar.dma_start(xT_sb[:, 6:KC, QT:T], xT[:, 6:KC, QT:T])
    nc.gpsimd.dma_start(ve_sb[:], ve3[:])
    nc.gpsimd.dma_start(mask_sb[:], masks[:])
    nc.gpsimd.dma_start(Wp_sb[:], Wp[:])
    nc.gpsimd.memset(magic_sb[:], MAGIC_RSQRT)

    # persistent intermediates
    kT = work.tile([128, NT, 128], BF16)          # transposed k (dup rows)
    v_g = work.tile([128, NT, 65], BF16)          # v + ones column
    qT = [work.tile([128, 2, QT], BF16, name=f"qT{j}") for j in range(NQ)]
    yT = [work.tile([128, 2, QT], BF16, name=f"yT{j}") for j in range(NQ)]
    ss_q = [work.tile([128, NQT, 5], F32, name=f"ss{j}") for j in range(NQ)]
    ssc = [work.tile([128, NQT, 5], F32, name=f"ssc{j}") for j in range(NQ)]
    g_q = [work.tile([128, NQT], F32, name=f"gq{j}") for j in range(NQ)]

    nc.gpsimd.memset(v_g[:, :, 64:65], 1.0)

    ps = stack.enter_context(tc.tile_pool(name="ps", bufs=1, space="PSUM"))
    sbp = stack.enter_context(tc.tile_pool(name="sbp", bufs=1))

    spans = {}
    for q0 in range(0, T, QT):
        sp = block_spans(q0, window, T)
        sp.sort(key=lambda s: (-(s[2] - s[1]), s[1]))
        spans[q0] = sp

    # assign Schraudolph (DVE) exp to full blocks, preferring later q-tiles
    # (ACT is busiest when attention is widest)
    schraud = set()
    budget = SCHRAUD_N
    for q0 in sorted(spans, reverse=True):
        for (kb, L, R, tcau, twin) in spans[q0]:
            if budget <= 0:
                break
            if tcau is None and twin is None:
                for pair in range(2):
                    if budget > 0:
                        schraud.add((q0, kb, pair))
                        budget -= 1

    nw_tmp = [None]

    def newton_rsqrt(mq, outq):
        """outq = 1/sqrt(mq) elementwise on [128, 20] f32 (DVE only)."""
        if nw_tmp[0] is None:
            nw_tmp[0] = [work.tile([128, 20], F32, name=f"nw{i}")
                         for i in range(3)]
        y0, y2, tt_ = nw_tmp[0]
        nc.vector.tensor_scalar(out=y0[:].bitcast(I32), in0=mq.bitcast(I32),
                                scalar1=1, scalar2=None,
                                op0=OP.arith_shift_right)
        nc.vector.tensor_tensor(y0[:].bitcast(I32), magic_sb[:].bitcast(I32),
                                y0[:].bitcast(I32), OP.subtract)
        for _ in range(2):
            nc.vector.tensor_tensor(y2[:], y0[:], y0[:], OP.mult)
            nc.vector.tensor_tensor(tt_[:], mq, y2[:], OP.mult)
            nc.vector.tensor_scalar(out=tt_[:], in0=tt_[:], scalar1=-0.5,
                                    scalar2=1.5, op0=OP.mult, op1=OP.add)
            nc.vector.tensor_tensor(y0[:], y0[:], tt_[:], OP.mult)
        nc.vector.tensor_copy(out=outq, in_=y0[:])

    # ---- task closures; emission order is interleaved so the PE always
    # ---- has independent filler work between dependent attention blocks.
    rp_keep = {}   # ti -> (qkv_sb, rp)

    def proj_tile(jq, tl):
        ti = jq * NQT + tl
        qkv_ps = ps.tile([128, 512], F32, tag="b1", bufs=2, name="qkv_ps")
        for k in range(KC):
            nc.tensor.matmul(
                qkv_ps[:, 0:386],
                xT_sb[:, k, ts(ti, 128)],
                W_sb[:, k, :],
                start=(k == 0), stop=(k == KC - 1),
            )
        qkv_sb = sbp.tile([128, 385], BF16, tag="qkvsb", bufs=6,
                          name="qkv_sb")
        nc.vector.tensor_copy(out=qkv_sb[:], in_=qkv_ps[:, 0:385])
        nc.scalar.activation(g_q[jq][:, tl:tl + 1], qkv_sb[:, 384:385],
                             AF.Exp, scale=-1.0)

        qk = qkv_sb[:, 0:320].rearrange("p (h d) -> p h d", d=64)
        cos_b = cos_sb[:, ti, None, :].to_broadcast((128, 5, 32))
        sin_b = sin_sb[:, ti, None, :].to_broadcast((128, 5, 32))
        rp = sbp.tile([128, 5, 64], BF16, tag="rope", bufs=6, name="rp")
        t1 = sbp.tile([128, 5, 64], BF16, tag="ropetmp", bufs=2, name="t1")
        nc.vector.tensor_tensor(rp[:, :, 0:32], qk[:, :, 0:32], cos_b,
                                OP.mult)
        nc.vector.tensor_tensor(rp[:, :, 32:64], qk[:, :, 32:64], cos_b,
                                OP.mult)
        nc.vector.tensor_tensor(t1[:, :, 0:32], qk[:, :, 32:64], sin_b,
                                OP.mult)
        nc.vector.tensor_tensor(t1[:, :, 32:64], qk[:, :, 0:32], sin_b,
                                OP.mult)
        nc.gpsimd.tensor_tensor(rp[:, :, 0:32], rp[:, :, 0:32],
                                t1[:, :, 0:32], OP.add)
        nc.gpsimd.tensor_tensor(rp[:, :, 32:64], rp[:, :, 32:64],
                                t1[:, :, 32:64], OP.subtract)
        sq = sbp.tile([128, 5, 64], BF16, tag="sq", bufs=2, name="sq")
        nc.vector.tensor_tensor(sq[:], rp[:], rp[:], OP.mult)
        nc.vector.tensor_reduce(ss_q[jq][:, tl, :], sq[:],
                                mybir.AxisListType.X, OP.add)
        rp_keep[ti] = (qkv_sb, rp)

    def epi_head(jq):
        # rms scale via fast-inverse-sqrt + gate sigmoid for the quarter
        mq = sbp.tile([128, 20], F32, tag="mq", bufs=2, name="mq")
        nc.vector.tensor_scalar(
            out=mq[:], in0=ss_q[jq][:].rearrange("p a b -> p (a b)"),
            scalar1=1.0 / (64.0 * RMS_MUL * RMS_MUL),
            scalar2=LN_EPS / (RMS_MUL * RMS_MUL), op0=OP.mult, op1=OP.add)
        nc.vector.reciprocal(mq[:], mq[:])
        nc.scalar.activation(ssc[jq][:].rearrange("p a b -> p (a b)"),
                             mq[:], AF.Sqrt)
        nc.vector.tensor_scalar(out=g_q[jq][:], in0=g_q[jq][:], scalar1=1.0,
                                scalar2=None, op0=OP.add)
        nc.vector.reciprocal(g_q[jq][:], g_q[jq][:])

    def epi_tile(jq, tl):
        ti = jq * NQT + tl
        qkv_sb, rp = rp_keep.pop(ti)
        nc.vector.scalar_tensor_tensor(
            out=v_g[:, ti, 0:64], in0=ve_sb[:, ti, :],
            scalar=g_q[jq][:, tl:tl + 1], in1=qkv_sb[:, 320:384],
            op0=OP.mult, op1=OP.add)
        qkn = sbp.tile([128, 5, 64], BF16, tag="qkn", bufs=2, name="qkn")
        nc.gpsimd.tensor_tensor(
            qkn[:], rp[:],
            ssc[jq][:, tl, :, None].to_broadcast((128, 5, 64)), OP.mult)
        tr = ps.tile([128, 1024], BF16, tag="b1", bufs=2, name="tr")
        nc.tensor.transpose(
            tr[:, 0:128], qkn[:, 0:2, :].rearrange("p h d -> p (h d)"),
            id_sb[:])
        nc.tensor.transpose(
            tr[:, 128:256], qkn[:, 2:4, :].rearrange("p h d -> p (h d)"),
            id_sb[:])
        nc.tensor.transpose(tr[0:64, 256:384], qkn[:, 4, :], id_sb[:])
        nc.tensor.transpose(tr[64:128, 256:384], qkn[:, 4, :], id_sb[:])
        nc.scalar.copy(
            out=qT[jq][:, :, ts(tl, 128)],
            in_=tr[:, 0:256].rearrange("p (h t) -> p h t", t=128))
        nc.scalar.copy(out=kT[:, ti, :], in_=tr[:, 256:384])

    y_live = {}    # pair -> [y_ps h0, y_ps h1]

    def attn_block(jq, pair, bi):
        q0 = jq * QT
        sp = spans[q0]
        nblk = len(sp)
        kb, L, R, tcau, twin = sp[bi]
        if bi == 0:
            y_live[pair] = [ps.tile([128, QT], F32, tag="y", bufs=2,
                                    name=f"y{h}") for h in range(2)]
        y_ps = y_live[pair]
        s_ps = ps.tile([128, 2, QT], F32, tag="s", bufs=2, name="s_ps")
        for h in range(2):
            rows = slice(64 * h, 64 * (h + 1))
            nc.tensor.matmul(
                s_ps[:, h, L:R],
                kT[rows, kb, :],
                qT[jq][rows, pair, L:R],
                start=True, stop=True,
                tile_position=(64 * h, 0),
            )
        p_sb = sbp.tile([128, 2, QT], BF16, tag="p", bufs=4, name="p_sb")
        if (q0, kb, pair) in schraud:
            nc.vector.tensor_scalar(
                out=p_sb[:, :, L:R].bitcast(I16),
                in0=s_ps[:, :, L:R], scalar1=ASCH, scalar2=BSCH,
                op0=OP.mult, op1=OP.add)
        else:
            nc.scalar.activation(p_sb[:, :, L:R], s_ps[:, :, L:R],
                                 AF.Exp, scale=0.125)
        for trip, moff_base in ((tcau, 0), (twin, 128)):
            if trip is None:
                continue
            c0, j0, w = trip
            # window triangles on the lightly-loaded gpsimd engine,
            # causal ones on DVE (all operands SBUF-resident)
            eng = nc.gpsimd if moff_base == 128 else nc.vector
            eng.tensor_tensor(
                p_sb[:, :, c0:c0 + w],
                p_sb[:, :, c0:c0 + w],
                mask_sb[:, None, moff_base + j0:moff_base + j0 + w]
                .to_broadcast((128, 2, w)),
                OP.mult)
        for h in range(2):
            nc.tensor.matmul(
                y_ps[h][0:65, L:R],
                v_g[:, kb, :],
                p_sb[:, h, L:R],
                start=(bi == 0),
                stop=(bi == nblk - 1),
            )

    def norm_pair(jq, pair):
        y_ps = y_live.pop(pair)
        for h in range(2):
            row_sb = sbp.tile([1, QT], F32, tag="srow", bufs=2,
                              name="row_sb")
            nc.scalar.copy(out=row_sb[:], in_=y_ps[h][64:65, :])
            r_sb = sbp.tile([1, QT], F32, tag="rrow", bufs=2, name="r_sb")
            nc.vector.reciprocal_approx_fast(out=r_sb[:], in_=row_sb[:])
            rbc = sbp.tile([64, QT], F32, tag="rbc", bufs=2, name="rbc")
            nc.gpsimd.partition_broadcast(rbc[:], r_sb[:])
            nc.vector.tensor_tensor(
                yT[jq][64 * h:64 * (h + 1), pair, :],
                y_ps[h][0:64, :], rbc[:], OP.mult)

    def outp_half(jq, tl, n):
        ti = jq * NQT + tl
        tsl = ts(tl, 128)
        nsl = ts(n, 512)
        o_ps = ps.tile([128, 512], F32, tag="b1", bufs=2, name="o_ps")
        nc.tensor.matmul(o_ps[:], yT[jq][:, 0, tsl],
                         Wp_sb[:, 0, nsl], start=True, stop=False)
        nc.tensor.matmul(o_ps[:], yT[jq][:, 1, tsl],
                         Wp_sb[:, 1, nsl], start=False, stop=True)
        o_sb = sbp.tile([128, 512], BF16, tag="osb", bufs=3, name="o_sb")
        if n == 0:
            nc.scalar.copy(out=o_sb[:], in_=o_ps[:])
        else:
            nc.vector.tensor_copy(out=o_sb[:], in_=o_ps[:])
        nc.sync.dma_start(out[ti][:, nsl], o_sb[:])

    # ---- emission schedule: proj runs two quarters ahead, epi one ----
    for tl in range(NQT):
        proj_tile(0, tl)
    for tl in range(NQT):
        proj_tile(1, tl)
    epi_head(0)
    for tl in range(NQT):
        epi_tile(0, tl)

    for jq in range(NQ):
        sp = spans[jq * QT]
        blocks = [(pair, bi) for pair in range(2) for bi in range(len(sp))]
        fillers = []
        for tl in range(NQT):
            if jq + 1 < NQ and tl == 0:
                fillers.append(lambda: epi_head(jq + 1))
            if jq + 1 < NQ:
                fillers.append(lambda tl=tl: epi_tile(jq + 1, tl))
            if jq > 0:
                fillers.append(lambda tl=tl: outp_half(jq - 1, tl, 0) or
                               outp_half(jq - 1, tl, 1))
        if jq + 2 < NQ:
            for tl in range(NQT):
                fillers.append(lambda tl=tl: proj_tile(jq + 2, tl))
        # distribute fillers evenly across the block stream
        fi = 0
        nb, nf = len(blocks), len(fillers)
        for i, (pair, bi) in enumerate(blocks):
            attn_block(jq, pair, bi)
            if bi == len(sp) - 1:
                norm_pair(jq, pair)
            want = (i + 1) * nf // nb
            while fi < want:
                fillers[fi]()
                fi += 1
        while fi < len(fillers):
            fillers[fi]()
            fi += 1
    for tl in range(NQT):
        outp_half(NQ - 1, tl, 0)
        outp_half(NQ - 1, tl, 1)

    stack.close()


@functools.lru_cache(maxsize=4)
def _build(window):
    nc = bacc.Bacc("TRN2", target_bir_lowering=False, debug=False,
                   enable_asserts=False, num_devices=8)
    ins = {
        "xT": nc.dram_tensor("xT", [C, T], BF16, kind="ExternalInput").ap(),
        "W_all": nc.dram_tensor("W_all", [C, 386], BF16,
                                kind="ExternalInput").ap(),
        "ve3": nc.dram_tensor("ve3", [T, 64], BF16, kind="ExternalInput").ap(),
        "cos": nc.dram_tensor("cos", [T, 32], BF16, kind="ExternalInput").ap(),
        "sin": nc.dram_tensor("sin", [T, 32], BF16, kind="ExternalInput").ap(),
        "Wproj": nc.dram_tensor("Wproj", [256, 1024], BF16,
                                kind="ExternalInput").ap(),
        "masks": nc.dram_tensor("masks", [128, 256], BF16,
                                kind="ExternalInput").ap(),
        "ident": nc.dram_tensor("ident", [128, 128], BF16,
                                kind="ExternalInput").ap(),
    }
    outs = {
        "out": nc.dram_tensor("out", [T, C], BF16, kind="ExternalOutput").ap(),
    }
    with tile.TileContext(nc) as tc:
        emit_kernel(tc, outs, ins, window)
    nc.compile()
    return nc


def host_constants():
    m_c = (np.arange(KB)[:, None] <= np.arange(KB)[None, :]).astype(np.float32)
    m_w = (np.arange(KB)[:, None] >= np.arange(KB)[None, :]).astype(np.float32)
    masks = np.concatenate([m_c, m_w], axis=1)
    ident = np.eye(128, dtype=np.float32)
    return masks, ident


def _bf(a):
    return np.asarray(a, dtype=ml_dtypes.bfloat16)


def prep_core_inputs(core, x, ve, cos, sin, Wq, Wk, Wv, Wproj, Wg):
    """Host-side shard prep for one core. core = b*4 + g."""
    b, g = divmod(core, NKV)
    masks, ident = host_constants()
    W_allf = np.zeros((C, 386), np.float32)
    W_allf[:, 0:256] = Wq[:, g * 256:(g + 1) * 256]
    W_allf[:, 256:320] = Wk[:, g * 64:(g + 1) * 64]
    W_allf[:, 320:384] = Wv[:, g * 64:(g + 1) * 64]
    W_allf[:12, 384] = Wg[:, g]
    return {
        "xT": _bf(np.ascontiguousarray(x[b].T)),
        "W_all": _bf(W_allf),
        "ve3": _bf(3.0 * ve[b, :, g * 64:(g + 1) * 64]),
        "cos": _bf(cos.reshape(T, 32)),
        "sin": _bf(sin.reshape(T, 32)),
        "Wproj": _bf(np.ascontiguousarray(Wproj[g * 256:(g + 1) * 256, :])),
        "masks": _bf(masks),
        "ident": _bf(ident),
    }


TRACE = False
LAST_EXEC_NS = None
LAST_TRACE = None


def kernel(x, ve, cos, sin, Wq, Wk, Wv, Wproj, Wg, window_size):
    x = np.asarray(x, np.float32)
    ve = np.asarray(ve, np.float32)
    cos = np.asarray(cos, np.float32)
    sin = np.asarray(sin, np.float32)
    Wq = np.asarray(Wq, np.float32)
    Wk = np.asarray(Wk, np.float32)
    Wv = np.asarray(Wv, np.float32)
    Wproj = np.asarray(Wproj, np.float32)
    Wg = np.asarray(Wg, np.float32)
    window = int(np.asarray(window_size))

    nc = _build(window)
    in_maps = [
        prep_core_inputs(core, x, ve, cos, sin, Wq, Wk, Wv, Wproj, Wg)
        for core in range(8)
    ]
    res = bass_utils.run_bass_kernel_spmd(nc, in_maps, core_ids=list(range(8)),
                                          trace=TRACE)
    if TRACE:
        global LAST_EXEC_NS, LAST_TRACE
        LAST_EXEC_NS = res.exec_time_ns
        LAST_TRACE = res
    out = np.zeros((B, T, C), np.float32)
    for core in range(8):
        out[core // NKV] += np.asarray(res.results[core]["out"], np.float32)
    return out

